# revision 3
# baseline (speedup 1.0000x reference)
"""Multi-head attention forward (B=2, S=2048, D=1024, H=16, Hd=64) on 8
Trainium2 NeuronCores — v2: bf16 datapath.

Sharding: core c handles batch b = c // 4 and the 4 heads (c % 4)*4 .. +4.
Each core computes its heads' Q/K/V projections, attention, and a partial
row-parallel o_proj; the host sums the 4 partial outputs per batch and adds
the output bias plus the (bv @ Wo) term (bv is dropped on-device; probs sum
to 1 so its contribution is exactly bv @ Wo).

Differences vs v1 (f32r everywhere):
  - All activations/weights stream in as bf16: input DMA halves, SBUF
    footprint halves. bf16 matmul runs at the same PE rate as f32r but
    with cheaper weight loads.
  - ex (exp of scores) is stored bf16: half the SBUF traffic.
  - The softmax 1/Z broadcast runs as bf16 [1,64]x[1,512] matmuls into a
    stacked [128, SBLK] psum (vs slow f32 in v1); 1/Z via DVE reciprocal
    mid-stream (emitted copies-first so the ctx psum frees quickly) and
    via ACT exp(-ln Z) on the tail unit only -- an Ln before the last
    exp would force the slower combined ln+exp activation table onto
    all 128 exps (+25us ACT).
  - ctx evac stacks both heads into one [128, SBLK] tile -> single
    normalize multiply per (qb, g), reading the broadcast psum directly.
  - K/V/Q-proj and o_proj work is spread across the 128-step attention
    stream in <=2us chunks placed to balance PE load against the ACT
    exp pace (ps_s double-buffering couples them 2 steps apart); LAG=6
    keeps a unit's first ctx clear of the previous unit's evac copies.
  - Output y is bf16 (halves output DMA); host sums partials in f32.
"""

import numpy as np
import ml_dtypes

S = 2048
D = 1024
H = 16
HD = 64
B = 2

P = 128
SBLK = 512          # s-positions per block
NSB = S // SBLK     # 4
DCH = D // P        # 8
NKT = S // P        # 16 key tiles
NH = 4              # heads per core
NG = 2              # head-pair groups per core
DLOC = NH * HD      # 256

_program_cache = {}


def _split_excess_waits(nc, mybir, max_waits=1):
    """This walrus build rejects instructions with >1 semaphore wait. Move
    excess waits onto preceding NoOps on the same engine queue (engines are
    strict FIFO, so blocking a NoOp blocks the instruction)."""
    n = 0
    for f in nc.m.functions:
        for bb in f.blocks:
            new = []
            changed = False
            for inst in bb.instructions:
                si = inst.sync_info
                waits = list(si.on_wait) if si is not None else []
                if len(waits) > max_waits:
                    extra = waits[:-max_waits]
                    keep = waits[-max_waits:]
                    for i in range(0, len(extra), max_waits):
                        nop = mybir.InstNoOp(
                            name=f"__waitsplit_{n}", ins=[], outs=[]
                        )
                        n += 1
                        nop.engine = inst.engine
                        nop.sync_info = mybir.SyncInfo(
                            on_wait=extra[i : i + max_waits], on_update=[]
                        )
                        new.append(nop)
                    inst.sync_info = mybir.SyncInfo(
                        on_wait=keep, on_update=list(si.on_update)
                    )
                    changed = True
                new.append(inst)
            if changed:
                bb.instructions = new
    return n


def _build_program():
    import concourse.bass as bass
    import concourse.mybir as mybir
    from concourse.bass import ds, ts
    from concourse.tile import TileContext

    f32 = mybir.dt.float32
    bf16 = mybir.dt.bfloat16
    AF = mybir.ActivationFunctionType

    nc = bass.Bass()
    xqT = nc.declare_dram_parameter("xqT", [D, S], bf16, isOutput=False)
    xkT = nc.declare_dram_parameter("xkT", [D, S], bf16, isOutput=False)
    xvT = nc.declare_dram_parameter("xvT", [D, S], bf16, isOutput=False)
    wq = nc.declare_dram_parameter("wq", [D, DLOC], bf16, isOutput=False)
    wk = nc.declare_dram_parameter("wk", [D, DLOC], bf16, isOutput=False)
    wv = nc.declare_dram_parameter("wv", [D, DLOC], bf16, isOutput=False)
    wo = nc.declare_dram_parameter("wo", [DLOC, D], bf16, isOutput=False)
    bq = nc.declare_dram_parameter("bq", [DLOC], f32, isOutput=False)
    bk = nc.declare_dram_parameter("bk", [DLOC], f32, isOutput=False)
    y = nc.declare_dram_parameter("y", [S, D], bf16, isOutput=True)

    with TileContext(nc) as tc:
        with (
            tc.tile_pool(name="const", bufs=1) as const,
            tc.tile_pool(name="kv", bufs=1) as kv,
            tc.tile_pool(name="xstr", bufs=3) as xstr,
            tc.tile_pool(name="epool", bufs=8) as epool,
            tc.tile_pool(name="cpool", bufs=2) as cpool,
            tc.tile_pool(name="upool", bufs=2) as upool,
            tc.tile_pool(name="rpool", bufs=2) as rpool,
            tc.tile_pool(name="opool", bufs=2) as opool,
            tc.tile_pool(name="ps_k", bufs=2, space="PSUM") as ps_k,
            tc.tile_pool(name="ps_s", bufs=2, space="PSUM") as ps_s,
            tc.tile_pool(name="ps_c", bufs=2, space="PSUM") as ps_c,
        ):
            # ---- warmup: trip the PE HAM activity monitor to full clock
            # while the first DMAs stream in.
            warm_in = const.tile([P, P], bf16)
            nc.gpsimd.memset(warm_in, 0.0)
            warm_ps = ps_k.tile([P, SBLK], f32, tag="k", name="warm")
            for _ in range(24):
                nc.tensor.matmul(
                    warm_ps[:, 0:P], warm_in[:], warm_in[:],
                    start=True, stop=True,
                )

            # ---- constants ------------------------------------------------
            one_sb = const.tile([P, 1], f32)
            nc.gpsimd.memset(one_sb, 1.0)
            # ones row for broadcasting a [1, SBLK] 1/Z row onto 64
            # head-dim partitions via a tiny matmul
            ones_f = const.tile([1, 64], f32)
            nc.gpsimd.memset(ones_f, 1.0)
            ones64 = const.tile([1, 64], bf16)
            nc.vector.tensor_copy(ones64[:], ones_f[:])

            # K/V path loads first: they gate the first scores
            wk_sb = const.tile([P, DCH, DLOC], bf16, name="w_k")
            nc.sync.dma_start(
                wk_sb[:], wk.rearrange("(dc p) n -> p dc n", p=P)
            )
            wv_sb = const.tile([P, DCH, DLOC], bf16, name="w_v")
            nc.sync.dma_start(
                wv_sb[:], wv.rearrange("(dc p) n -> p dc n", p=P)
            )
            bk_sb = const.tile([P, NG], f32)
            nc.sync.dma_start(bk_sb[:], bk.rearrange("(g p) -> p g", p=P))
            # wq/bq DMAs are emitted after emit_kv(0) below so xk0/xv0
            # (which gate the first scores) head the DMA queue
            wq_sb = const.tile([P, DCH, DLOC], bf16, name="w_q")
            bq_sb = const.tile([P, NG], f32)
            # wo is not needed until the first o_proj; its DMA is emitted
            # inside the attention stream
            wo_sb = const.tile([P, NG, D], bf16)

            # persistent Q/K/V state
            qhT = kv.tile([P, NG, S], bf16)
            khT = [
                kv.tile([P, S], bf16, tag=f"khT{g}", name=f"khT{g}")
                for g in range(NG)
            ]
            # vh_aug: [sk-part, kt, head*65] with col 64 of each head == 1.0
            vh_aug = kv.tile([P, NKT, NH * 65], bf16)
            vh4 = vh_aug[:].rearrange("p k (h e) -> p k h e", e=65)
            nc.vector.tensor_copy(
                vh4[:, :, :, 64], one_sb[:].to_broadcast([P, NKT, NH])
            )

            def stream_xT(dram, blk, name):
                t = xstr.tile([P, DCH, SBLK], bf16, tag=name, name=name)
                view = dram.rearrange("(dc p) s -> p dc s", p=P)
                for dc in range(DCH):
                    nc.sync.dma_start(
                        t[:, dc], view[:, dc, ds(blk * SBLK, SBLK)]
                    )
                return t

            def emit_qproj_g(xq_blk, qb, g):
                pq = ps_k.tile([P, SBLK], f32, tag="k", name="pq")
                for dc in range(DCH):
                    nc.tensor.matmul(
                        pq[:],
                        wq_sb[:, dc, ts(g, P)],
                        xq_blk[:, dc, :],
                        start=(dc == 0),
                        stop=(dc == DCH - 1),
                    )
                nc.vector.tensor_scalar_add(
                    qhT[:, g, ts(qb, SBLK)], pq[:], bq_sb[:, g : g + 1]
                )

            def emit_kproj_g(xk_blk, sb, g):
                pk = ps_k.tile([P, SBLK], f32, tag="k", name="pk")
                for dc in range(DCH):
                    nc.tensor.matmul(
                        pk[:],
                        wk_sb[:, dc, ts(g, P)],
                        xk_blk[:, dc, :],
                        start=(dc == 0),
                        stop=(dc == DCH - 1),
                    )
                nc.vector.tensor_scalar_add(
                    khT[g][:, ts(sb, SBLK)], pk[:], bk_sb[:, g : g + 1]
                )

            def emit_vproj_ss(xv_blk, sb, ss):
                pv = ps_k.tile([P, DLOC], f32, tag="k", name="pv")
                for dc in range(DCH):
                    nc.tensor.matmul(
                        pv[:],
                        xv_blk[:, dc, ts(ss, P)],
                        wv_sb[:, dc, :],
                        start=(dc == 0),
                        stop=(dc == DCH - 1),
                    )
                kt = sb * 4 + ss
                nc.vector.tensor_copy(
                    vh4[:, kt, :, 0:64],
                    pv[:].rearrange("p (h e) -> p h e", e=64),
                )

            def emit_kv(sb):
                xk_blk = stream_xT(xkT, sb, "xk")
                for g in range(NG):
                    emit_kproj_g(xk_blk, sb, g)
                xv_blk = stream_xT(xvT, sb, "xv")
                for ss in range(4):
                    emit_vproj_ss(xv_blk, sb, ss)

            # ---- attention + o_proj: one continuous pipeline -------------
            LAG = 6
            pcs = {}
            exs = {}
            ctx2s = {}
            ctxus = {}
            rcs = {}

            def emit_scores_exp(qb, g, kt):
                ps2 = ps_s.tile([P, 2, SBLK], f32, tag="s", name="ps2")
                for hh in range(2):
                    hr = hh * 64
                    nc.tensor.matmul(
                        ps2[:, hh, :],
                        khT[g][hr : hr + 64, ts(kt, P)],
                        qhT[hr : hr + 64, g, ts(qb, SBLK)],
                        start=True,
                        stop=True,
                        tile_position=(hr, 0),
                    )
                ex = epool.tile([P, 2, SBLK], bf16, name="ex")
                nc.scalar.activation(ex[:], ps2[:], AF.Exp, scale=0.125)
                exs[(qb, g, kt)] = ex

            def emit_ctx(qb, g, kt):
                if kt == 0:
                    pcs[(qb, g)] = [
                        ps_c.tile([65, SBLK], f32, tag="c", name=f"pc{hh}")
                        for hh in range(2)
                    ]
                    if g == 0:
                        ctx2s[qb] = cpool.tile(
                            [P, NG, SBLK], bf16, name="ctx2"
                        )
                ex = exs.pop((qb, g, kt))
                for hh in range(2):
                    h = 2 * g + hh
                    nc.tensor.matmul(
                        pcs[(qb, g)][hh][:],
                        vh_aug[:, kt, h * 65 : h * 65 + 65],
                        ex[:, hh, :],
                        start=(kt == 0),
                        stop=(kt == NKT - 1),
                    )

            def emit_evac(qb, g):
                # Evacuate the finished ctx psum pair to SBUF (stacked into
                # one [128, SBLK] tile) and the Z rows into per-head
                # [1, SBLK] tiles; 1/Z via reciprocal_approx_fast (18-bit).
                u2 = upool.tile([P, SBLK], f32, tag="u", name="u")
                # All psum-draining copies FIRST: the next unit's first ctx
                # (one step later) reuses these psum slots, so nothing slow
                # may sit between the copies in the DVE queue. The 3.3us
                # reciprocals run after. In-stream Ln/Exp is NOT an option:
                # any Ln before the last exp forces the slower combined
                # ln+exp activation table onto all 128 exps (+25us ACT).
                tail = (qb, g) == (NSB - 1, NG - 1)
                zrs = []
                for hh in range(2):
                    pc = pcs[(qb, g)][hh]
                    nc.vector.tensor_copy(
                        u2[hh * 64 : hh * 64 + 64, :], pc[0:64, :]
                    )
                    if tail:
                        # ACT is past its exps; Ln straight from psum and
                        # exp(-ln Z) skip the DVE reciprocal and its queue
                        rcb = rpool.tile([1, SBLK], bf16, tag=f"rb{hh}",
                                         name="rcb")
                        lz = rpool.tile([1, SBLK], f32, tag=f"lz{hh}",
                                        name="lz")
                        nc.scalar.activation(lz[:], pc[64:65, :], AF.Ln)
                        nc.scalar.activation(
                            rcb[:], lz[:], AF.Exp, scale=-1.0
                        )
                        rcs[(qb, g, hh)] = rcb
                    else:
                        zr = rpool.tile([1, SBLK], f32, tag=f"z{hh}",
                                        name="zr")
                        nc.vector.tensor_copy(zr[:], pc[64:65, :])
                        zrs.append(zr)
                del pcs[(qb, g)]
                ctxus[(qb, g)] = u2
                for hh in range(2):
                    if tail:
                        break
                    rc = rpool.tile([1, SBLK], f32, tag=f"r{hh}",
                                    name="rc")
                    nc.vector.reciprocal(rc[:], zrs[hh][:])
                    rcb = rpool.tile([1, SBLK], bf16, tag=f"rb{hh}",
                                     name="rcb")
                    nc.vector.tensor_copy(rcb[:], rc[:])
                    rcs[(qb, g, hh)] = rcb

            def emit_norm_rest(qb, g):
                ctx2 = ctx2s[qb]
                u2 = ctxus.pop((qb, g))
                pb = ps_s.tile([P, 1, SBLK], f32, tag="s", name="pb")
                for hh in range(2):
                    nc.tensor.matmul(
                        pb[hh * 64 : hh * 64 + 64, 0, :],
                        ones64[:],
                        rcs.pop((qb, g, hh))[:],
                        start=True,
                        stop=True,
                    )
                nc.vector.tensor_mul(ctx2[:, g, :], u2[:], pb[:, 0, :])

            # K/V blocks, q-block projections, o_proj and norms are spread
            # across the step stream in small chunks (<= ~2us of PE work)
            # so the ACT exp stream never stalls behind a PE burst and the
            # PE never waits long on an ex tile.
            plan = {}

            def at(i, fn):
                plan.setdefault(i, []).append(fn)

            def make_kv(sb, d, s0):
                st = {}

                def dma():
                    st["xk"] = stream_xT(xkT, sb, "xk")
                    st["xv"] = stream_xT(xvT, sb, "xv")

                at(d, dma)
                for g in range(NG):
                    at(s0 + g,
                       lambda g=g: emit_kproj_g(st["xk"], sb, g))
                for vp in range(2):
                    def vchunk(vp=vp):
                        emit_vproj_ss(st["xv"], sb, 2 * vp)
                        emit_vproj_ss(st["xv"], sb, 2 * vp + 1)
                    at(s0 + NG + vp, vchunk)

            def make_qproj(qb, d, s0):
                st = {}

                def dma():
                    st["xq"] = stream_xT(xqT, qb, "xq")

                at(d, dma)
                for g in range(NG):
                    at(s0 + g,
                       lambda g=g: emit_qproj_g(st["xq"], qb, g))

            def emit_o_proj_qs(qb, qs):
                ctx2 = ctx2s[qb]
                ost = opool.tile([P, D], bf16, name="ost")
                for nch in range(2):
                    po = ps_k.tile([P, SBLK], f32, tag="k", name="po")
                    for g in range(NG):
                        nc.tensor.matmul(
                            po[:],
                            ctx2[:, g, ts(qs, P)],
                            wo_sb[:, g, ts(nch, SBLK)],
                            start=(g == 0),
                            stop=(g == NG - 1),
                        )
                    nc.vector.tensor_copy(ost[:, ts(nch, SBLK)], po[:])
                nc.sync.dma_start(y[ds(qb * SBLK + qs * P, P), :], ost[:])
                if qs == 3:
                    del ctx2s[qb]

            make_kv(1, d=0, s0=2)
            at(3, lambda: nc.sync.dma_start(
                wo_sb[:], wo.rearrange("(g p) n -> p g n", p=P)))
            make_kv(2, d=2, s0=6)
            make_kv(3, d=5, s0=10)
            make_qproj(1, d=10, s0=18)
            make_qproj(2, d=50, s0=56)
            make_qproj(3, d=82, s0=88)
            for oqb in range(NSB - 1):
                for qs in range(4):
                    at((oqb + 1) * 2 * NKT + 14 + qs + (qs >= 2),
                       lambda oqb=oqb, qs=qs: emit_o_proj_qs(oqb, qs))

            emit_kv(0)
            nc.sync.dma_start(
                wq_sb[:], wq.rearrange("(dc p) n -> p dc n", p=P)
            )
            nc.sync.dma_start(bq_sb[:], bq.rearrange("(g p) -> p g", p=P))
            xq0 = stream_xT(xqT, 0, "xq")
            for g in range(NG):
                emit_qproj_g(xq0, 0, g)
            steps = [
                (qb, g, kt)
                for qb in range(NSB)
                for g in range(NG)
                for kt in range(NKT)
            ]
            for i, (qb, g, kt) in enumerate(steps):
                emit_scores_exp(qb, g, kt)
                for fn in plan.get(i, ()):
                    fn()
                if i >= LAG:
                    pqb, pg, pkt = steps[i - LAG]
                    emit_ctx(pqb, pg, pkt)
                    if pkt == NKT - 1:
                        emit_evac(pqb, pg)
                if kt == 13:
                    if g == 1:
                        emit_norm_rest(qb, 0)
                    elif qb > 0:
                        emit_norm_rest(qb - 1, 1)
            for j in range(len(steps) - LAG, len(steps)):
                qb, g, kt = steps[j]
                emit_ctx(qb, g, kt)
                if kt == NKT - 1:
                    emit_evac(qb, g)

            # fused tail for the last unit: normalize and o_proj pipeline
            # per 128-column chunk
            qb, g = NSB - 1, 1
            ctx2 = ctx2s.pop(qb)
            u2 = ctxus.pop((qb, g))
            pb = ps_s.tile([P, 1, SBLK], f32, tag="s", name="pb")
            for hh in range(2):
                nc.tensor.matmul(
                    pb[hh * 64 : hh * 64 + 64, 0, :],
                    ones64[:],
                    rcs.pop((qb, g, hh))[:],
                    start=True,
                    stop=True,
                )
            for qs in range(4):
                qsl = ts(qs, P)
                nc.vector.tensor_mul(
                    ctx2[:, g, qsl], u2[:, qsl], pb[:, 0, qsl]
                )
                ost = opool.tile([P, D], bf16, name="ost")
                for nch in range(2):
                    po = ps_k.tile([P, SBLK], f32, tag="k", name="po")
                    for gg in range(NG):
                        nc.tensor.matmul(
                            po[:],
                            ctx2[:, gg, qsl],
                            wo_sb[:, gg, ts(nch, SBLK)],
                            start=(gg == 0),
                            stop=(gg == NG - 1),
                        )
                    nc.vector.tensor_copy(ost[:, ts(nch, SBLK)], po[:])
                nc.sync.dma_start(y[ds(qb * SBLK + qs * P, P), :], ost[:])

    _split_excess_waits(nc, mybir)
    return nc


def kernel(q, k, v, Wq, bq, Wk, bk, Wv, bv, Wo, bo):
    from concourse.bass_utils import run_bass_kernel_spmd

    BF = ml_dtypes.bfloat16

    q = np.asarray(q, dtype=np.float32)
    k = np.asarray(k, dtype=np.float32)
    v = np.asarray(v, dtype=np.float32)
    Wq = np.asarray(Wq, dtype=np.float32)
    Wk = np.asarray(Wk, dtype=np.float32)
    Wv = np.asarray(Wv, dtype=np.float32)
    Wo = np.asarray(Wo, dtype=np.float32)
    bq = np.asarray(bq, dtype=np.float32)
    bk = np.asarray(bk, dtype=np.float32)
    bv = np.asarray(bv, dtype=np.float32)
    bo = np.asarray(bo, dtype=np.float32)

    if "nc" not in _program_cache:
        _program_cache["nc"] = _build_program()
    nc = _program_cache["nc"]

    qT = [np.ascontiguousarray(q[b].T).astype(BF) for b in range(B)]
    kT = [np.ascontiguousarray(k[b].T).astype(BF) for b in range(B)]
    vT = [np.ascontiguousarray(v[b].T).astype(BF) for b in range(B)]

    in_maps = []
    for c in range(8):
        b, hg = c // 4, c % 4
        cols = slice(DLOC * hg, DLOC * (hg + 1))
        in_maps.append(
            {
                "xqT": qT[b],
                "xkT": kT[b],
                "xvT": vT[b],
                "wq": np.ascontiguousarray(Wq[:, cols]).astype(BF),
                "wk": np.ascontiguousarray(Wk[:, cols]).astype(BF),
                "wv": np.ascontiguousarray(Wv[:, cols]).astype(BF),
                "wo": np.ascontiguousarray(Wo[cols, :]).astype(BF),
                "bq": np.ascontiguousarray(bq[cols]),
                "bk": np.ascontiguousarray(bk[cols]),
            }
        )

    global _last_in_maps
    _last_in_maps = in_maps

    res = run_bass_kernel_spmd(nc, in_maps, list(range(8)))

    out = np.empty((B, S, D), np.float32)
    bvwo = (bv @ Wo).astype(np.float32)
    for b in range(B):
        acc = res.results[4 * b]["y"].astype(np.float32)
        for hg in range(1, 4):
            acc = acc + res.results[4 * b + hg]["y"].astype(np.float32)
        out[b] = acc + bvwo[None, :] + bo[None, :]
    return out
